# revision 61
# baseline (speedup 1.0000x reference)
"""Multi-head attention (B=2, S=2048, D=1024, H=16) on 8 TRN2 NeuronCores.

Sharding: batch x head-group. Core c handles batch c//4 and heads
[4*(c%4), 4*(c%4)+4). Each core computes its heads' Q/K/V projections
(column-parallel), causal attention, and a row-parallel partial of the
output projection. The host sums the 4 bf16 partials per batch
(all-reduce done on host during unshard) and adds dense_b.

All streaming data is bf16 (halves HBM traffic and keeps every matmul at
1 PE-cycle/row regardless of free-size); accumulation stays in fp32
PSUM. On-core dataflow (transposed, feature-major):
  QT = WqT.T @ XqT   [256, 2048]   (bf16 matmuls, K=1024 in 8 chunks)
  KT, V likewise (V in natural [S, 256] layout, + ones column for row sums)
  per head pair, per 512-wide q-block, per 128-wide k-chunk:
    logitsT [sk=128, sq] = KT_chunk.T @ QT_block   (2 heads row-packed
      at tile_position (0,0)/(64,0), shared 2-bank PSUM tile); for
      diagonal chunks only columns [off:512] are computed (causal clip)
    PT = exp(0.125 * logitsT) -> bf16     (ScalarE, PSUM->SBUF)
    diagonal chunks: multiply PT[off:off+128] by 0/1 upper-tri (DVE)
    OT[65, off:512] += V_aug.T @ PT    (V_aug = [V | ones] -> rows 0:64 =
                                        O^T, row 64 = softmax denominator)
  per head pair, after the last AV: softmax normalization entirely OFF
    the PE (the baseline used a K=1 matmul broadcast): the sums row is
    staged to an offset-0 SBUF tile (plain DVE copies remap partition
    bases; the custom ops only honor offset 0), then rc =
    reciprocal_approx_fast (DVE), rc broadcast across 64 partitions with
    GPSIMD partition_broadcast, OT_norm = OT * rc (DVE; head-odd shifted
    to partitions 64:128 by GPSIMD). The staging copies are emitted in
    the AV-flush shadow; the rc/broadcast/multiply pieces are dripped one
    per chunk (prioq) into the next pair's attention so the DVE FIFO
    never clogs ahead of the fill evacs that recycle the mmp PSUM ring.
  dense partial outT[., q-block] += denseT.T @ OT_norm, evacuated in
    mc-pairs sharing one HBM DMA
Causality: fully-masked k-chunks are skipped and diagonal chunks are
column-clipped (~2x less work). A generic path (any mask) adds
maskT * -8e9 to every chunk instead.

Schedule (the attention loop is exp-paced: ScalarE ~1040ns per k-chunk
vs ~850ns of PE work, so every other engine's work is drip-fed into it):
 - startup DMAs are interleaved (first weight chunk, then x quarters,
   wk/wv woven into the xq/xk streams) so the first projection matmul
   issues at ~2.5us and the PE is never waiting on a tensor switch;
 - A0 runs up front (ACT free -> evacs may use it), then B0's FIRST
   head pair (it needs only A0's outputs, and fills the window where A1
   would stall on the x1 DMAs), then A1; A2/A3's qk-groups fill
   B0pc1/B1; their V-GROUPS fill B2/B3 themselves (V_j is first read by
   B_j's diagonal AV chunks, so it may land mid-block) — this feeds the
   late, exp-paced blocks that would otherwise starve;
 - dense(j) drips into block j+1 via a low-priority queue, a few groups
   held back for the eager tail;
 - AV matmuls trail their logits by av_depth chunks (software pipeline);
 - eager tail: in the last block's second head pair, AVs flush as soon
   as their exp lands, so O's columns finalize progressively; the
   normalization chain, dense, evac and half-DMAs then run per 128-col
   slice overlapped with the remaining diagonal chunks, leaving only
   ~5us of drain after the last matmul.
"""

import numpy as np
import ml_dtypes
from contextlib import ExitStack

import concourse.tile as tile
from concourse import bacc, mybir
from concourse.bass_utils import run_bass_kernel_spmd

F32 = mybir.dt.float32
BF16 = mybir.dt.bfloat16
AF = mybir.ActivationFunctionType
ADD = mybir.AluOpType.add
MULT = mybir.AluOpType.mult

NPBF = ml_dtypes.bfloat16

B, S, D, H = 2, 2048, 1024, 16
NCORES = 8
HL = 4            # heads per core
DH = D // H       # 64
DLOC = HL * DH    # 256 local feature dims
SBK = 512         # seq block (q)
NSB = S // SBK    # 4
KCH = 128         # k chunk


def _ts(i, n):
    return slice(i * n, (i + 1) * n)


def build(causal=True, with_bq=False, with_bk=False, with_bv=False,
          x_bufs=2, pt_bufs=9, ev_bufs=6, small_bufs=4, ot_bufs=4,
          av_depth=7, eager_tail=True, warm2=False, quota_on=True,
          a3_split=5, hold_n=5, flush_mod=1, qnum=1, qden=3,
          chain_drip=1, b0_early=1):
    nc = bacc.Bacc(None, target_bir_lowering=False)

    xqT = nc.dram_tensor("xqT", [D, S], BF16, kind="ExternalInput")
    xkT = nc.dram_tensor("xkT", [D, S], BF16, kind="ExternalInput")
    xvT = nc.dram_tensor("xvT", [D, S], BF16, kind="ExternalInput")
    wqT = nc.dram_tensor("wqT", [D, DLOC], BF16, kind="ExternalInput")
    wkT = nc.dram_tensor("wkT", [D, DLOC], BF16, kind="ExternalInput")
    wvT = nc.dram_tensor("wvT", [D, DLOC], BF16, kind="ExternalInput")
    dnT = nc.dram_tensor("dnT", [DLOC, D], BF16, kind="ExternalInput")
    if not causal:
        mskT = nc.dram_tensor("mskT", [S, S], F32, kind="ExternalInput")
    bq = nc.dram_tensor("bq", [DLOC], BF16, kind="ExternalInput") if with_bq else None
    bk = nc.dram_tensor("bk", [DLOC], BF16, kind="ExternalInput") if with_bk else None
    bv = nc.dram_tensor("bv", [DLOC], BF16, kind="ExternalInput") if with_bv else None
    outT = nc.dram_tensor("outT", [D, S], BF16, kind="ExternalOutput")

    any_bias = with_bq or with_bk or with_bv
    # packed constants, one DMA: col 0 = ones column (V_aug softmax-sum
    # trick), cols 1:129 = upper (inclusive) triangle tri01[r,c] = r <= c
    cpack_np = np.zeros((128, 1 + KCH), np.float32)
    cpack_np[:, 0] = 1.0
    cpack_np[:, 1:] = np.triu(np.ones((KCH, KCH), np.float32))
    cpack = nc.inline_tensor(cpack_np.astype(NPBF), name="cpack")
    if any_bias:
        ones512 = nc.inline_tensor(np.ones((1, 512), NPBF), name="ones512")

    with tile.TileContext(nc) as tc, ExitStack() as ctx:
        pers = ctx.enter_context(tc.tile_pool(name="pers", bufs=1))
        xpool = ctx.enter_context(tc.tile_pool(name="xpool", bufs=x_bufs))
        ptp = ctx.enter_context(tc.tile_pool(name="ptp", bufs=pt_bufs))
        otp = ctx.enter_context(tc.tile_pool(name="otp", bufs=ot_bufs))
        evp = ctx.enter_context(tc.tile_pool(name="evp", bufs=ev_bufs))
        smallp = ctx.enter_context(tc.tile_pool(name="smallp", bufs=small_bufs))
        if not causal:
            mskp = ctx.enter_context(tc.tile_pool(name="mskp", bufs=3))
        mmp = ctx.enter_context(tc.tile_pool(name="mmp", bufs=2, space="PSUM"))
        lp = ctx.enter_context(tc.tile_pool(name="lp", bufs=2, space="PSUM"))
        opp = ctx.enter_context(tc.tile_pool(name="opp", bufs=1, space="PSUM"))

        # ---------- persistent tiles ----------
        wsb = {}
        for wname in ("q", "k", "v"):
            wsb[wname] = pers.tile([128, 8, DLOC], BF16, tag=f"w{wname}",
                                   name=f"w_{wname}")
        dn_sb = pers.tile([128, 2, D], BF16, tag="dn")
        cpk = pers.tile([128, 1 + KCH], BF16, tag="cpk")
        if any_bias:
            ones_r = pers.tile([1, 512], BF16, tag="ones_r")

        QT_sb = pers.tile([128, 2, S], BF16, tag="QT")
        KT_sb = pers.tile([128, 2, S], BF16, tag="KT")
        V_sb = pers.tile([128, S // KCH, HL, DH + 1], BF16, tag="V")

        xdram = {"q": xqT, "k": xkT, "v": xvT}
        wdram = {"q": wqT, "k": wkT, "v": wvT}

        # ---------- startup DMAs: block-0 x + weights, interleaved so the
        # first projection matmul can issue as early as possible ----------
        js0 = _ts(0, SBK)
        xt0 = {}
        wre = {}
        srce = {}
        for xname in ("q", "k", "v"):
            wre[xname] = wdram[xname].rearrange("(c p) m -> p c m", p=128)
            srce[xname] = xdram[xname].rearrange("(c p) s -> p c s", p=128)
            xt0[xname] = xpool.tile([128, 8, SBK], BF16, tag=f"x{xname}",
                                    name=f"x_{xname}_0")
        # DMA issue order woven so each projection group's weights land
        # just ahead of its x stream and the PE is never waiting on a
        # tensor switch: wq/xq interleaved, wk inside the xq stream, wv
        # inside the xk stream. The first LDWEIGHTS needs only wq[kc=0].
        startup = (
            ("w", "q", 0, 1), ("x", "q", 0, 2), ("w", "q", 1, 4),
            ("x", "q", 2, 4), ("w", "k", 0, 4), ("w", "q", 4, 8),
            ("x", "q", 4, 6), ("x", "q", 6, 8), ("w", "k", 4, 8),
            ("x", "k", 0, 2), ("x", "k", 2, 4), ("w", "v", 0, 4),
            ("x", "k", 4, 6), ("x", "k", 6, 8), ("w", "v", 4, 8),
            ("x", "v", 0, 2), ("x", "v", 2, 4), ("x", "v", 4, 6),
            ("x", "v", 6, 8),
        )
        for kind, xname, a, b in startup:
            if kind == "w":
                nc.sync.dma_start(out=wsb[xname][:, a:b, :],
                                  in_=wre[xname][:, a:b, :])
            else:
                nc.sync.dma_start(out=xt0[xname][:, a:b, :],
                                  in_=srce[xname][:, a:b, js0])
        nc.sync.dma_start(out=cpk, in_=cpack[:, :])

        bsb = {}
        for name, dram in (("q", bq), ("k", bk), ("v", bv)):
            if dram is not None:
                t = pers.tile([1, DLOC], BF16, tag=f"b{name}")
                nc.sync.dma_start(out=t, in_=dram[None, :])
                bsb[name] = t
        if any_bias:
            nc.sync.dma_start(out=ones_r, in_=ones512[:, :])

        # ones column of V_aug (softmax denominator trick)
        nc.vector.tensor_copy(
            V_sb[:, :, :, DH:DH + 1],
            cpk[:, None, None, 0:1].broadcast_to([128, S // KCH, HL, 1]),
        )

        def load_x(j, js):
            xt = {}
            for xname in ("q", "k", "v"):
                srcr = xdram[xname].rearrange("(c p) s -> p c s", p=128)
                t = xpool.tile([128, 8, SBK], BF16, tag=f"x{xname}",
                               name=f"x_{xname}_{j}")
                # two-way split so the first consuming matmuls can start
                # halfway through the block's transfer
                nc.sync.dma_start(out=t[:, 0:4, :], in_=srcr[:, 0:4, js])
                nc.sync.dma_start(out=t[:, 4:8, :], in_=srcr[:, 4:8, js])
                xt[xname] = t
            return xt

        outT_r = outT.rearrange("(c p) s -> p c s", p=128)

        # round-robin copy engines for PSUM evacuation (ACT is reserved for
        # exp during attention; phase-A-only copies may use it)
        def copier(engines=("vector",), _state={}):
            k = engines
            i = _state.get(k, 0)
            _state[k] = i + 1
            eng = getattr(nc, engines[i % len(engines)])

            def cp(out, in_):
                if hasattr(eng, "tensor_copy"):
                    eng.tensor_copy(out, in_)
                else:
                    eng.copy(out=out, in_=in_)
            return type("C", (), {"tensor_copy": staticmethod(cp)})

        def phase_A_groups(j, js, xt, evac=("vector",)):
            # ---------- projections for s-block j, as 8 independent
            # matmul-group thunks so they can be interleaved into phase B ----
            def qk_group(bname, dst, mc):
                def emit():
                    ps = mmp.tile([128, 512], F32, tag="mm")
                    has_b = bname in bsb
                    for kc in range(8):
                        nc.tensor.matmul(
                            ps[:, :],
                            lhsT=wsb[bname][:, kc, _ts(mc, 128)],
                            rhs=xt[bname][:, kc, :],
                            start=(kc == 0), stop=(kc == 7 and not has_b),
                        )
                    if has_b:
                        nc.tensor.matmul(
                            ps[:, :], lhsT=bsb[bname][0:1, _ts(mc, 128)],
                            rhs=ones_r[0:1, 0:SBK], start=False, stop=True,
                        )
                    copier(evac).tensor_copy(dst[:, mc, js], ps)
                return emit

            def v_group(sc):
                def emit():
                    ps = mmp.tile([128, 512], F32, tag="mm")
                    has_b = "v" in bsb
                    for kc in range(8):
                        nc.tensor.matmul(
                            ps[:, 0:DLOC],
                            lhsT=xt["v"][:, kc, _ts(sc, 128)],
                            rhs=wsb["v"][:, kc, :],
                            start=(kc == 0), stop=(kc == 7 and not has_b),
                        )
                    if has_b:
                        nc.tensor.matmul(
                            ps[:, 0:DLOC], lhsT=ones_r[0:1, 0:128],
                            rhs=bsb["v"][0:1, :], start=False, stop=True,
                        )
                    copier(evac).tensor_copy(
                        V_sb[:, j * 4 + sc, :, 0:DH],
                        ps[:, 0:DLOC].rearrange("p (h d) -> p h d", h=HL),
                    )
                return emit

            return ([qk_group(b, d, mc) for b, d in (("q", QT_sb), ("k", KT_sb))
                     for mc in range(2)] + [v_group(sc) for sc in range(4)])

        def emit_denorm(j, pc, O, OTs):
            # softmax normalization, entirely off the PE. NOTE: the
            # custom-DVE reciprocal and Pool partition_broadcast only honor
            # partition offset 0 on their inputs, and Pool cannot read PSUM —
            # so stage the sums row (O row 64) to an offset-0 SBUF tile with
            # a plain DVE copy (plain copies DO remap partition bases), then
            # run the custom ops at offset 0.
            # The sums staging is emitted immediately (the AV-flush region
            # has DVE slack); the reciprocal/broadcast/multiply pieces are
            # queued on `prioq` and dripped one per chunk into the next head
            # pair's attention so the DVE FIFO never clogs ahead of the fill
            # evacs (which the mmp ring needs to keep the PE fed).
            otpair = otp.tile([128, 512], BF16, tag=f"otp{pc}",
                              name=f"otp_{j}_{pc}")
            OTs[pc] = otpair
            sm = [None, None]
            for i in (1, 0):
                t = smallp.tile([1, 512], F32, tag=f"sm{i}",
                                name=f"sm_{j}_{pc}_{i}")
                nc.vector.tensor_copy(t, O[i][64:65, :])
                sm[i] = t
            rcb = [None, None]

            def t_rc(i):
                def emit():
                    rc = smallp.tile([1, 512], F32, tag=f"rc{i}",
                                     name=f"rc_{j}_{pc}_{i}")
                    nc.vector.reciprocal_approx_fast(out=rc, in_=sm[i])
                    rcb[i] = smallp.tile([64, 512], F32, tag=f"rcb{i}",
                                         name=f"rcb_{j}_{pc}_{i}")
                    nc.gpsimd.partition_broadcast(rcb[i], rc, channels=64)
                return emit

            def t_mul0():
                nc.vector.tensor_tensor(out=otpair[0:64, :],
                                        in0=O[0][0:64, :], in1=rcb[0],
                                        op=MULT)

            def t_mul1():
                ott = smallp.tile([64, 512], BF16, tag="ott",
                                  name=f"ott_{j}_{pc}")
                nc.vector.tensor_tensor(out=ott, in0=O[1][0:64, :],
                                        in1=rcb[1], op=MULT)
                # partition shift 0:64 -> 64:128 (gpsimd remaps partitions
                # SBUF->SBUF without the DMA latency chain)
                nc.gpsimd.tensor_copy(otpair[64:128, :], ott)

            if chain_drip == 2:
                # finer drip: one DVE op per piece
                def t_r(i):
                    def emit():
                        rc = smallp.tile([1, 512], F32, tag=f"rc{i}",
                                         name=f"rc_{j}_{pc}_{i}")
                        nc.vector.reciprocal_approx_fast(out=rc, in_=sm[i])
                        sm[i] = rc
                    return emit

                def t_p(i):
                    def emit():
                        rcb[i] = smallp.tile([64, 512], F32, tag=f"rcb{i}",
                                             name=f"rcb_{j}_{pc}_{i}")
                        nc.gpsimd.partition_broadcast(rcb[i], sm[i],
                                                      channels=64)
                    return emit
                prioq.extend([t_r(0), t_p(0), t_r(1), t_p(1), t_mul0, t_mul1])
            elif chain_drip == 0 and j == NSB - 2 and pc == 1:
                # emit immediately (no drip) for the chain feeding the
                # eager tail region
                for fn in (t_rc(0), t_rc(1), t_mul0, t_mul1):
                    fn()
            else:
                prioq.extend([t_rc(0), t_rc(1), t_mul0, t_mul1])

        def emit_denorm_col(j, pc, O, otpair, c):
            # 128-column slice of the normalization chain, used by the
            # eager tail: sums staged on ACT (it has slack between the short
            # diagonal exps), reciprocal/multiplies on DVE, broadcast/shift
            # on Pool
            cs = slice(128 * c, 128 * (c + 1))
            rcb = [None, None]
            for i in (1, 0):
                t = smallp.tile([1, 512], F32, tag=f"sm{i}",
                                name=f"smc_{j}_{pc}_{i}_{c}")
                nc.scalar.copy(out=t[:, 0:KCH], in_=O[i][64:65, cs])
                rc = smallp.tile([1, 512], F32, tag=f"rc{i}",
                                 name=f"rcc_{j}_{pc}_{i}_{c}")
                nc.vector.reciprocal_approx_fast(out=rc[:, 0:KCH],
                                                 in_=t[:, 0:KCH])
                rcb[i] = smallp.tile([64, 512], F32, tag=f"rcb{i}",
                                     name=f"rcbc_{j}_{pc}_{i}_{c}")
                nc.gpsimd.partition_broadcast(rcb[i][:, 0:KCH], rc[:, 0:KCH],
                                              channels=64)
            ott = smallp.tile([64, 512], BF16, tag="ott",
                              name=f"ottc_{j}_{pc}_{c}")
            nc.vector.tensor_tensor(out=ott[:, 0:KCH], in0=O[1][0:64, cs],
                                    in1=rcb[1][:, 0:KCH], op=MULT)
            nc.vector.tensor_tensor(out=otpair[0:64, cs], in0=O[0][0:64, cs],
                                    in1=rcb[0][:, 0:KCH], op=MULT)
            nc.gpsimd.tensor_copy(otpair[64:128, cs], ott[:, 0:KCH])

        def dense_thunks(j, js, OTs):
            box = {}

            def grp(mc):
                def emit():
                    dps = mmp.tile([128, 512], F32, tag="mm",
                                   name=f"dps_{j}_{mc}")
                    for pc in range(2):
                        nc.tensor.matmul(
                            dps[:, :], lhsT=dn_sb[:, pc, _ts(mc, 128)],
                            rhs=OTs[pc][:, :], start=(pc == 0), stop=(pc == 1),
                        )
                    # pair consecutive mc's into one ev tile / one out-DMA
                    # (halves the 625ns-per-DMA HWDGE serialization)
                    if mc % 2 == 0:
                        box["ev"] = evp.tile([128, 2, 512], BF16, tag="ev",
                                             name=f"ev_{j}_{mc}")
                    ev = box["ev"]
                    copier().tensor_copy(ev[:, mc % 2, :], dps)
                    if mc % 2:
                        nc.sync.dma_start(
                            out=outT_r[:, mc - 1:mc + 1, js],
                            in_=ev)
                return emit
            return [grp(mc) for mc in range(8)]

        def tail_dense_col(js, OTs, c, evs, dcnt):
            # 128-column slice of the final block's dense: two PSUM views
            # hold all 8 mc outputs for this column; one whole evac per view
            # (alternating DVE/ACT per column to balance); one [128,4,256]
            # half-DMA per ev tile when its half completes. Columns 2-3 use
            # freed L-pool banks so they are not ring-coupled to columns
            # 0-1's evacs (which sit deep in the ACT/DVE FIFOs).
            dps = [mmp.tile([128, 4, KCH], F32, tag="mm",
                            name=f"dpst_{c}_{hf}") for hf in range(2)]
            for mc in range(8):
                for pc in range(2):
                    nc.tensor.matmul(
                        dps[mc // 4][:, mc % 4, :],
                        lhsT=dn_sb[:, pc, _ts(mc, 128)],
                        rhs=OTs[pc][:, 128 * c:128 * (c + 1)],
                        start=(pc == 0), stop=(pc == 1),
                    )
            for hf in range(2):
                dst = evs[hf][:, :, 128 * c:128 * (c + 1)]
                if (hf + c) % 2:
                    nc.scalar.copy(out=dst, in_=dps[hf])
                else:
                    nc.vector.tensor_copy(dst, dps[hf])
                dcnt[hf] += 1
                if dcnt[hf] in (2, 4):
                    hh = 0 if dcnt[hf] == 2 else 1
                    nc.sync.dma_start(
                        out=outT_r[:, 4 * hf:4 * hf + 4,
                                   (NSB - 1) * SBK + 256 * hh:
                                   (NSB - 1) * SBK + 256 * (hh + 1)],
                        in_=evs[hf][:, :, 256 * hh:256 * (hh + 1)])

        def phase_B(j, js, fill, lowq, pcs=(0, 1)):
            # ---------- attention + dense for q-block j ----------
            # `fill`: queue of thunks (phase-A groups via fill, dense groups
            # via lowq), popped into the exp-paced attention loop where the
            # PE would otherwise idle: one per chunk, two while the AV
            # software pipeline is still filling (those chunks have no AV).
            nkc = (j + 1) * 4 if causal else S // KCH

            def pop_fill(quota=None):
                # denorm continuation pieces first (no PE work, no quota)
                if prioq:
                    prioq.pop(0)()
                if quota is not None and quota[0] <= 0:
                    return
                if fill:
                    fill.pop(0)()
                elif lowq:
                    lowq.pop(0)()
                else:
                    return
                if quota is not None:
                    quota[0] -= 1

            OTs = OTS_ALL.setdefault(j, [None, None])
            for pc in pcs:
                # eager tail: for the very last head pair, flush AVs as soon
                # as their exp lands so O's columns finalize progressively,
                # and run the normalization + dense + evac + DMA per column
                # slice overlapped with the remaining diagonal chunks
                tailpc = causal and eager_tail and j == NSB - 1 and pc == 1
                # reserve roughly half the queued fill for the second head
                # pair so its AV-less warmup chunks are covered too
                avail = len(fill) + len(lowq)
                quota = [(avail * qnum + 1) // qden if pc == 0 else avail] if quota_on else None
                O = [
                    opp.tile([65, 512], F32, tag=f"o{i}", name=f"O_{j}_{pc}_{i}")
                    for i in range(2)
                ]
                if tailpc:
                    otpair = otp.tile([128, 512], BF16, tag=f"otp{pc}",
                                      name=f"otp_{j}_{pc}")
                    OTs[pc] = otpair
                    evs = [evp.tile([128, 4, 512], BF16, tag="evt",
                                    name=f"evt_{hf}") for hf in range(2)]
                    dcnt = [0, 0]
                pend = []  # software pipeline: AV trails logits by PD kc's

                def emit_av(kc, off, last, PT):
                    for i in range(2):
                        nc.tensor.matmul(
                            O[i][0:65, off:SBK],
                            lhsT=V_sb[:, kc, 2 * pc + i, :],
                            rhs=PT[:, i, off:SBK],
                            start=(kc == 0), stop=last,
                            skip_group_check=True,
                        )

                for kc in range(nkc):
                    diag = causal and kc >= 4 * j
                    off = (kc - 4 * j) * KCH if diag else 0
                    L = lp.tile([128, 2, SBK], F32, tag="L")
                    for i in range(2):
                        nc.tensor.matmul(
                            L[:, i, off:SBK],
                            lhsT=KT_sb[_ts(i, 64), pc, _ts(kc, KCH)],
                            rhs=QT_sb[_ts(i, 64), pc,
                                      j * SBK + off:(j + 1) * SBK],
                            start=True, stop=True,
                            tile_position=(64 * i, 0),
                        )
                    pop_fill(quota)
                    if tailpc and kc >= 4 * j:
                        for _ in range(2):
                            if hold2:
                                hold2.pop(0)()
                    if warm2 and len(pend) < av_depth:
                        # AV pipeline still filling: the PE has only the two
                        # logits matmuls this chunk — pop a second thunk
                        pop_fill(quota)
                    if not causal:
                        mk = mskp.tile([128, SBK], F32, tag="mk")
                        nc.sync.dma_start(out=mk, in_=mskT[_ts(kc, KCH), js])
                        nc.vector.tensor_tensor(
                            out=L[:, :, :], in0=L[:, :, :],
                            in1=mk[:, None, :].broadcast_to([128, 2, SBK]),
                            op=ADD,
                        )
                    PT = ptp.tile([128, 2, SBK], BF16, tag="PT")
                    nc.scalar.activation(
                        out=PT[:, :, off:SBK], in_=L[:, :, off:SBK],
                        func=AF.Exp, scale=0.125)
                    if diag:
                        # triangle mask as 0/1 multiply (off the exp edge)
                        nc.vector.tensor_tensor(
                            out=PT[:, :, off:off + KCH],
                            in0=PT[:, :, off:off + KCH],
                            in1=cpk[:, None, 1:1 + KCH].broadcast_to(
                                [128, 2, KCH]),
                            op=MULT,
                        )
                    pend.append((kc, off, kc == nkc - 1, PT))
                    if tailpc and kc >= 4 * j:
                        while pend:
                            emit_av(*pend.pop(0))
                        emit_denorm_col(j, pc, O, otpair, kc - 4 * j)
                        if kc > 4 * j:
                            # dense for the column completed last iteration
                            # (its chain has had a full chunk to resolve);
                            # emitted after this chunk's exp so its ACT evac
                            # copies don't delay the exp cadence
                            tail_dense_col(js, OTs, kc - 4 * j - 1, evs, dcnt)
                    elif len(pend) > av_depth:
                        emit_av(*pend.pop(0))
                for fi, p_ in enumerate(pend):
                    # absorb each flushed AV's exp-wait with leftover work
                    if fi % flush_mod == 0:
                        pop_fill()
                    emit_av(*p_)
                if tailpc:
                    tail_dense_col(js, OTs, 3, evs, dcnt)
                else:
                    emit_denorm(j, pc, O, OTs)
            if j < NSB - 1 and 1 in pcs:
                # dense is latency-tolerant (ot_bufs=4 removes the tile-ring
                # coupling): queue it for the next blocks' fill slots; part
                # of dense2 is held back to feed the eager tail chunks
                gs = dense_thunks(j, js, OTs)
                if causal and eager_tail and j == NSB - 2:
                    hold2.extend(gs[len(gs) - hold_n:])
                    gs = gs[:len(gs) - hold_n]
                lowq.extend(gs)
            elif not (causal and eager_tail):
                for g in dense_thunks(j, js, OTs):
                    g()

        # schedule: A0, A1 up front (PE runway; ACT free, so evacs may use
        # it), then B(j) with A(j+2)'s groups and earlier blocks' dense
        # drip-fed into the exp-paced attention loops.
        xts = {0: xt0}
        for g in phase_A_groups(0, _ts(0, SBK), xts.pop(0),
                                evac=("scalar", "vector")):
            g()
        xts[1] = load_x(1, _ts(1, SBK))
        nc.sync.dma_start(out=dn_sb,
                          in_=dnT.rearrange("(c p) n -> p c n", p=128))
        OTS_ALL = {}
        fill = []
        lowq = []
        prioq = []
        hold2 = []
        # B0 can run BETWEEN A0 and A1: its attention needs only A0's
        # outputs, and it fills the window where A1 stalls on the x1 DMAs
        if b0_early:
            phase_B(0, _ts(0, SBK), fill, lowq,
                    pcs=(0, 1) if b0_early == 2 else (0,))
        for g in phase_A_groups(1, _ts(1, SBK), xts.pop(1),
                                evac=("scalar", "vector")):
            g()
        xts[2] = load_x(2, _ts(2, SBK))
        xts[3] = load_x(3, _ts(3, SBK))
        # A(j)'s qk-groups must land before B(j)'s first logits, but its
        # v-groups are consumed only by B(j)'s DIAGONAL chunks (the last 4+sc
        # of each head pair) — so they can fill B(j)'s own early, fill-starved
        # chunks. Routing: B0 <- A2qk, B1 <- A3qk, B2 <- A2v, B3 <- A3v;
        # dense(j) drips into block j+1 via lowq.
        A2 = phase_A_groups(2, _ts(2, SBK), xts.pop(2))
        A3 = phase_A_groups(3, _ts(3, SBK), xts.pop(3))
        routes = {0: A2[:4], 1: A3[:4], 2: A2[4:], 3: A3[4:]}
        for j in range(NSB):
            fill.extend(routes[j])
            if j == 0:
                if b0_early == 2:
                    continue  # emitted already; A2qk flows into B1's fill
                phase_B(j, _ts(j, SBK), fill, lowq,
                        pcs=(1,) if b0_early else (0, 1))
            else:
                phase_B(j, _ts(j, SBK), fill, lowq)
        while fill:
            fill.pop(0)()
        while lowq:
            lowq.pop(0)()
        while prioq:
            prioq.pop(0)()
        while hold2:
            hold2.pop(0)()

    nc.finalize()
    return nc


_CACHE = {}


def _get_nc(causal, with_bq, with_bk, with_bv):
    key = (causal, with_bq, with_bk, with_bv)
    if key not in _CACHE:
        _CACHE[key] = build(causal, with_bq, with_bk, with_bv)
    return _CACHE[key]


def _bf(a):
    return np.ascontiguousarray(a).astype(NPBF)


def _prep_in_maps(query, key_, value, mask2d, causal, wq_w, wk_w, wv_w, dense_w,
                  wq_b, wk_b, wv_b, with_bq, with_bk, with_bv):
    in_maps = []
    xT = {}
    for b in range(B):
        xT[b] = (_bf(query[b].T), _bf(key_[b].T), _bf(value[b].T))
    mskT = None if causal else np.ascontiguousarray(mask2d.T * np.float32(-8e9))
    for c in range(NCORES):
        b, g = divmod(c, 4)
        sl = _ts(g, DLOC)
        m = {
            "xqT": xT[b][0], "xkT": xT[b][1], "xvT": xT[b][2],
            "wqT": _bf(wq_w[sl].T),
            "wkT": _bf(wk_w[sl].T),
            "wvT": _bf(wv_w[sl].T),
            "dnT": _bf(dense_w[:, sl].T),
        }
        if not causal:
            m["mskT"] = mskT
        if with_bq:
            m["bq"] = _bf(wq_b[sl])
        if with_bk:
            m["bk"] = _bf(wk_b[sl])
        if with_bv:
            m["bv"] = _bf(wv_b[sl])
        in_maps.append(m)
    return in_maps


def _run(in_maps, causal, with_bq, with_bk, with_bv, **kw):
    nc = _get_nc(causal, with_bq, with_bk, with_bv)
    return run_bass_kernel_spmd(nc, in_maps, core_ids=list(range(NCORES)), **kw)


def kernel(query, key_, value, mask, wq_w, wq_b, wk_w, wk_b, wv_w, wv_b,
           dense_w, dense_b, _profile_kw=None):
    query = np.asarray(query, np.float32)
    key_ = np.asarray(key_, np.float32)
    value = np.asarray(value, np.float32)
    mask2d = np.asarray(mask, np.float32).reshape(S, S)
    wq_w = np.asarray(wq_w, np.float32)
    wk_w = np.asarray(wk_w, np.float32)
    wv_w = np.asarray(wv_w, np.float32)
    dense_w = np.asarray(dense_w, np.float32)
    wq_b = np.asarray(wq_b, np.float32)
    wk_b = np.asarray(wk_b, np.float32)
    wv_b = np.asarray(wv_b, np.float32)
    dense_b = np.asarray(dense_b, np.float32)

    causal = bool(np.array_equal(mask2d, np.triu(np.ones((S, S), np.float32), k=1)))
    with_bq = bool(np.any(wq_b))
    with_bk = bool(np.any(wk_b))
    with_bv = bool(np.any(wv_b))

    in_maps = _prep_in_maps(query, key_, value, mask2d, causal, wq_w, wk_w, wv_w,
                            dense_w, wq_b, wk_b, wv_b, with_bq, with_bk, with_bv)
    res = _run(in_maps, causal, with_bq, with_bk, with_bv, **(_profile_kw or {}))

    out = np.empty((B, S, D), np.float32)
    for b in range(B):
        acc = res.results[4 * b]["outT"].astype(np.float32).copy()
        for g in range(1, 4):
            acc += res.results[4 * b + g]["outT"]
        out[b] = acc.T + dense_b[None, :]
    if _profile_kw:
        return out, res
    return out
